# revision 7
# baseline (speedup 1.0000x reference)
"""Trainium2 Bass kernel for batched differentiable mean-variance optimization.

Problem: for each of 256 samples, solve
    min 0.5 y^T Sigma y  s.t.  mu^T y = 1, y >= 0
via 150 unrolled projected-gradient iterations (step = 1/lambda_max via power
iteration), then normalize to portfolio weights.  Pure data parallel: 32
samples per core on 8 cores.

v2 design (vs v1 fp32r):
- Sigma resident in SBUF as bf16 for all 32 samples (16 MB).  bf16 matmul
  streams at 1 cyc/row and allows 4 concurrent samples in distinct 32-column
  groups of the PE array (tile_position) -> ~4x matvec throughput.
- Matvec psum output rows {32*db} for 2 quarter-sets share a [128,2,512] psum
  pair tile; one ScalarE copy [97,2,512] stages it, one DMA scatters 8
  samples into the A4 projection layout (partition pi = 4*slot + quarter).
- Projection (semismooth Newton on the simplex-like constraint) runs on DVE
  in per-half [64,128] A4 tiles; per-sample sums via a block-diag G matmul
  (sum over the 4 quarter-partitions of a sample + broadcast back).
- The two halves (16 samples each) are software-pipelined: half h's matvec
  streams on PE while half 1-h runs Newton on DVE; the small G matmuls are
  emitted interleaved between matvec groups so PE never waits long.
- Slot permutation: DRAM sample b = 8P + 4s + db lives at slot j = 8P+2db+s
  (pi = 4j + q).  Host-visible DMAs (mu in, w out) undo it with rearranged
  DRAM access patterns; sigma load permutes in the Python loop.
"""

import os
import numpy as np
from contextlib import ExitStack

N = 512
NCORES = 8
SPC = 32           # samples per core
POWER_ITERS = 8
PGD_ITERS = 150
NEWTON_K = 3

_PROGRAM_CACHE = {}


def _slot_to_dram(j):
    """x_B/sigma slot j -> DRAM sample row (per core)."""
    P, r = divmod(j, 8)
    db, s = divmod(r, 2)
    return 8 * P + 4 * s + db


def _build_program(power_iters=POWER_ITERS, pgd_iters=PGD_ITERS,
                   newton_k=NEWTON_K):
    import concourse.bacc as bacc
    import concourse.tile as tile
    from concourse import mybir

    Alu = mybir.AluOpType
    F32 = mybir.dt.float32
    BF16 = mybir.dt.bfloat16

    nc = bacc.Bacc(
        "TRN2",
        target_bir_lowering=False,
        debug=False,
        enable_asserts=False,
        num_devices=NCORES,
    )

    mu_dram = nc.dram_tensor("mu_in", [SPC, N], F32, kind="ExternalInput").ap()
    sig_dram = nc.dram_tensor("sigma_in", [SPC, N, N], BF16,
                              kind="ExternalInput").ap()
    g64_dram = nc.dram_tensor("g64_in", [64, 64], F32, kind="ExternalInput").ap()
    id64_dram = nc.dram_tensor("id64_in", [64, 64], F32, kind="ExternalInput").ap()
    w_dram = nc.dram_tensor("w_out", [SPC, N], F32, kind="ExternalOutput").ap()

    # DRAM sample b = 8P + 4s + d lives at A4 partitions 32P' + 8d + 4s + q
    # (q = 0..3 contiguous).  Host-visible transfers go one DMA per sample:
    # DRAM [1, 512] <-> A4 [4 contiguous partitions, 128].
    def a4_sample_range(Pp, s, dd):
        base = 32 * Pp + 8 * dd + 4 * s
        return base, base + 4

    def half_samples(h):
        for Pp in range(2):
            for s in range(2):
                for dd in range(4):
                    b = 8 * (2 * h + Pp) + 4 * s + dd
                    yield Pp, s, dd, b

    with tile.TileContext(nc) as tc, ExitStack() as ctx:
        const_pool = ctx.enter_context(tc.tile_pool(name="const", bufs=1))
        sig_pool = ctx.enter_context(tc.tile_pool(name="sig", bufs=1))
        state_pool = ctx.enter_context(tc.tile_pool(name="state", bufs=1))
        adma_pool = ctx.enter_context(tc.tile_pool(name="adma", bufs=2))
        mv_pool = ctx.enter_context(tc.tile_pool(name="mv", bufs=1, space="PSUM"))
        tr_pool = ctx.enter_context(tc.tile_pool(name="tr", bufs=1, space="PSUM"))
        nw_pool = ctx.enter_context(tc.tile_pool(name="nw", bufs=1, space="PSUM"))

        g64_sb = const_pool.tile([64, 64], F32)
        nc.sync.dma_start(out=g64_sb, in_=g64_dram)
        id64_sb = const_pool.tile([64, 64], F32)
        nc.sync.dma_start(out=id64_sb, in_=id64_dram)

        # Sigma resident: [part p, slot j, chunk c, elem e] = Sigma[b(j)][128c+p, e]
        sig_sb = sig_pool.tile([128, SPC, 4, N], BF16)
        for j in range(SPC):
            nc.sync.dma_start(
                out=sig_sb[:, j],
                in_=sig_dram[_slot_to_dram(j)].rearrange("(c p) e -> p c e", p=128),
            )

        # B layout iterate: partition = element within quarter, free (slot, q).
        x_B = state_pool.tile([128, SPC, 4], BF16, tag="xB")

        # Per-half A4 state ([64, 128]: partition 4*j_rel + q, free elem).
        # Free dim padded to 132 so DMA AP balancing can never merge the
        # per-partition row with the partition stride into runs that would
        # cross partitions (that merge is physically wrong on SBUF).
        H = {}
        for h in (0, 1):
            d = {}
            for nm in ("mu", "imu", "msq", "g", "ys", "u", "r", "muv", "t",
                       "yfin", "va4", "wa4"):
                d[nm] = state_pool.tile([64, 128], F32, tag=f"h{h}_{nm}",
                                        name=f"h{h}_{nm}",
                                        padded_shape=[64, 132])
            d["prod"] = state_pool.tile([64, 2, 128], F32, tag=f"h{h}_prod",
                                        name=f"h{h}_prod")
            for nm in ("ab", "nd"):
                d[nm] = state_pool.tile([64, 2], F32, tag=f"h{h}_{nm}",
                                        name=f"h{h}_{nm}")
            for nm in ("neglam", "lam", "rb", "bmax", "negstep", "invnegstep",
                       "cnt", "mvd", "omv", "sp", "ok", "sc", "off", "s2"):
                d[nm] = state_pool.tile([64, 1], F32, tag=f"h{h}_{nm}",
                                        name=f"h{h}_{nm}")
            H[h] = d

        if os.environ.get("KM_SIM_SAFE"):
            # The interpreter's init tracking can't merge scattered DMA
            # writes; pre-fill scatter targets (sim correctness aid only).
            for h in (0, 1):
                for nm in ("mu", "g", "va4", "wa4"):
                    nc.vector.memset(H[h][nm], 0.0)

        for h in (0, 1):
            d = H[h]
            for Pp, s, dd, b in half_samples(h):
                p0, p1 = a4_sample_range(Pp, s, dd)
                nc.sync.dma_start(out=d["mu"][p0:p1, :],
                                  in_=mu_dram[b: b + 1, :])
            nc.vector.reciprocal(d["imu"], d["mu"])
            nc.vector.tensor_mul(d["msq"], d["mu"], d["mu"])

        # ---------- matvec ----------
        def matvec_stages(h, dst):
            """Return 8 closures; running all emits the matvec of half h's 16
            slots into A4 tile `dst` ([64,128]).  Fine stages (8 MMs each,
            ~0.85us) give the scheduler slots to place Newton gmms and
            transposes where their inputs are long-ready, keeping the PE
            stream gap-free (p-state stays at max)."""
            stages = []
            tiles = {}
            for Pp in (0, 1):          # quarter-pair within the half
                P = 2 * h + Pp
                for si in range(4):
                    s_idx, p_pair = divmod(si, 2)

                    def mm_stage(P=P, Pp=Pp, s_idx=s_idx, p_pair=p_pair):
                        if s_idx == 0 and p_pair == 0:
                            ps = mv_pool.tile([128, 2, 512], F32,
                                              tag=f"mv{P % 2}", name=f"mv{P % 2}")
                            if os.environ.get("KM_SIM_SAFE"):
                                nc.vector.memset(ps, 0.0)
                            tiles[P] = ps
                        ps = tiles[P]
                        for p in (2 * p_pair, 2 * p_pair + 1):
                            for db in range(4):
                                j = 8 * P + 2 * db + s_idx
                                nc.tensor.matmul(
                                    ps[32 * db: 32 * db + 1, s_idx, :],
                                    x_B[:, j, p: p + 1],
                                    sig_sb[:, j, p, :],
                                    start=(p == 0),
                                    stop=(p == 3),
                                    tile_position=(0, 32 * db),
                                )
                        if s_idx == 1 and p_pair == 1:
                            stage = adma_pool.tile(
                                [128, 2, 512], F32, tag=f"st{P % 2}",
                                name=f"st{P % 2}", bufs=2)
                            nc.scalar.copy(stage[0:97], ps[0:97])
                            nc.sync.dma_start(
                                out=dst[32 * Pp: 32 * Pp + 32, :],
                                in_=stage[0:97:32].rearrange("d s f -> d (s f)"),
                            )
                    stages.append(mm_stage)
            return stages

        def tr_slice(h, Pp, src):
            """Transpose pair-slice Pp of half h's A4 tile back into x_B."""
            trp = tr_pool.tile([128, 32], F32, tag=f"tr{Pp}", name=f"trp{Pp}")
            nc.tensor.transpose(
                trp, src[32 * Pp: 32 * Pp + 32, :],
                id64_sb[32 * Pp: 32 * Pp + 32, 32 * Pp: 32 * Pp + 32],
                tile_position=(32 * Pp, 0),
            )
            nc.vector.tensor_copy(
                x_B[:, 16 * h + 8 * Pp: 16 * h + 8 * Pp + 8, :],
                trp.rearrange("p (j q) -> p j q", q=4),
            )

        def transpose_to_xB(h, src_a4):
            tr_slice(h, 0, src_a4)
            tr_slice(h, 1, src_a4)

        def gmm(h, rhs, out_ps, n):
            nc.tensor.matmul(out_ps[:, 0:n], g64_sb, rhs[:, 0:n],
                             start=True, stop=True)

        # ---------- Newton projection ----------
        def newton_stages(h, r_ap, muv_ap):
            """Closure list for newton_k iterations of the lam solve for half
            h.  Each gmm is its own stage so it can interleave with matvec
            matmuls on the PE queue."""
            d = H[h]
            stages = []
            for _ in range(newton_k):
                def dve_part(d=d, r_ap=r_ap, muv_ap=muv_ap):
                    nc.vector.scalar_tensor_tensor(
                        out=d["prod"][:, 0, :], in0=r_ap,
                        scalar=d["neglam"][:, 0:1], in1=muv_ap,
                        op0=Alu.is_gt, op1=Alu.mult, accum_out=d["ab"][:, 0:1],
                    )
                    nc.vector.scalar_tensor_tensor(
                        out=d["prod"][:, 1, :], in0=r_ap,
                        scalar=d["neglam"][:, 0:1], in1=d["msq"],
                        op0=Alu.is_gt, op1=Alu.mult, accum_out=d["ab"][:, 1:2],
                    )

                def pe_part(d=d, h=h):
                    abp = nw_pool.tile([64, 2], F32, tag=f"nw{h}", name=f"nw{h}")
                    gmm(h, d["ab"], abp, 2)
                    nc.vector.tensor_scalar(
                        out=d["bmax"], in0=abp[:, 1:2], scalar1=1e-30,
                        scalar2=None, op0=Alu.max,
                    )
                    nc.vector.reciprocal(d["rb"], d["bmax"])
                    nc.vector.scalar_tensor_tensor(
                        out=d["neglam"], in0=abp[:, 0:1], scalar=-1.0,
                        in1=d["rb"], op0=Alu.add, op1=Alu.mult,
                    )
                stages.append(dve_part)
                stages.append(pe_part)
            return stages

        def slice_update_stages(h, k, Pp):
            """Stage closures for pair-slice Pp (partitions 32Pp..32Pp+32) of
            half h, iteration k: [pre+stt1, g1+stt2, g2+stt3, g3+post].  The
            caller interleaves the gmm stages into the other half's matvec
            stream so the PE never waits on DVE."""
            d = H[h]
            sl = slice(32 * Pp, 32 * Pp + 32)

            def stt():
                nc.vector.scalar_tensor_tensor(
                    out=d["prod"][sl, 0, :], in0=d["r"][sl],
                    scalar=d["neglam"][sl, 0:1], in1=d["muv"][sl],
                    op0=Alu.is_gt, op1=Alu.mult, accum_out=d["ab"][sl, 0:1],
                )
                nc.vector.scalar_tensor_tensor(
                    out=d["prod"][sl, 1, :], in0=d["r"][sl],
                    scalar=d["neglam"][sl, 0:1], in1=d["msq"][sl],
                    op0=Alu.is_gt, op1=Alu.mult, accum_out=d["ab"][sl, 1:2],
                )

            def gmm_smalls():
                abp = nw_pool.tile([64, 2], F32, tag=f"nw{h}", name=f"nw{h}")
                nc.tensor.matmul(
                    abp[sl, 0:2], g64_sb[sl, 32 * Pp: 32 * Pp + 32],
                    d["ab"][sl, 0:2], start=True, stop=True,
                    tile_position=(32 * Pp, 32 * Pp),
                )
                nc.vector.tensor_scalar(
                    out=d["bmax"][sl], in0=abp[sl, 1:2], scalar1=1e-30,
                    scalar2=None, op0=Alu.max,
                )
                nc.vector.reciprocal(d["rb"][sl], d["bmax"][sl])
                nc.vector.scalar_tensor_tensor(
                    out=d["neglam"][sl], in0=abp[sl, 0:1], scalar=-1.0,
                    in1=d["rb"][sl], op0=Alu.add, op1=Alu.mult,
                )

            def pre():
                nc.vector.scalar_tensor_tensor(
                    out=d["u"][sl], in0=d["ys"][sl],
                    scalar=d["invnegstep"][sl, 0:1], in1=d["g"][sl],
                    op0=Alu.mult, op1=Alu.add,
                )
                nc.vector.tensor_mul(d["r"][sl], d["u"][sl], d["imu"][sl])
                nc.vector.tensor_mul(d["muv"][sl], d["u"][sl], d["mu"][sl])
                stt()

            def post():
                nc.vector.tensor_scalar(
                    out=d["lam"][sl], in0=d["neglam"][sl], scalar1=-1.0,
                    scalar2=None, op0=Alu.mult,
                )
                nc.vector.scalar_tensor_tensor(
                    out=d["t"][sl], in0=d["mu"][sl], scalar=d["lam"][sl, 0:1],
                    in1=d["u"][sl], op0=Alu.mult, op1=Alu.add,
                )
                if k < pgd_iters - 1:
                    nc.vector.tensor_scalar(
                        out=d["ys"][sl], in0=d["t"][sl], scalar1=0.0,
                        scalar2=d["negstep"][sl, 0:1], op0=Alu.max, op1=Alu.mult,
                    )
                else:
                    nc.vector.tensor_scalar(
                        out=d["yfin"][sl], in0=d["t"][sl], scalar1=0.0,
                        scalar2=None, op0=Alu.max,
                    )

            stages = [pre]
            for j in range(newton_k):
                last = (j == newton_k - 1)

                def g_st(last=last):
                    gmm_smalls()
                    if not last:
                        stt()
                    else:
                        post()
                stages.append(g_st)
            return stages

        # ---------- power phase (pipelined halves, gap-free PE) ----------
        nc.vector.memset(x_B, 1.0)
        pend_pow = None   # half whose va4 awaits its transpose back to x_B
        for kk in range(power_iters):
            for h in (0, 1):
                mv = matvec_stages(h, H[h]["va4"])
                mv[0]()
                if pend_pow is not None:
                    tr_slice(pend_pow, 0, H[pend_pow]["va4"])
                mv[1]()
                mv[2]()
                if pend_pow is not None:
                    tr_slice(pend_pow, 1, H[pend_pow]["va4"])
                    pend_pow = None
                for i in range(3, 8):
                    mv[i]()
                pend_pow = h
        if pend_pow is not None:
            tr_slice(pend_pow, 0, H[pend_pow]["va4"])
            tr_slice(pend_pow, 1, H[pend_pow]["va4"])
            pend_pow = None
        for h in (0, 1):
            for st in matvec_stages(h, H[h]["wa4"]):
                st()
        for h in (0, 1):
            d = H[h]
            nc.vector.scalar_tensor_tensor(
                out=d["prod"][:, 0, :], in0=d["va4"], scalar=0.0, in1=d["wa4"],
                op0=Alu.add, op1=Alu.mult, accum_out=d["nd"][:, 0:1],
            )
            nc.vector.scalar_tensor_tensor(
                out=d["prod"][:, 1, :], in0=d["va4"], scalar=0.0, in1=d["va4"],
                op0=Alu.add, op1=Alu.mult, accum_out=d["nd"][:, 1:2],
            )
            nwp = nw_pool.tile([64, 2], F32, tag=f"nw{h}", name=f"nw{h}")
            gmm(h, d["nd"], nwp, 2)
            nc.vector.reciprocal(d["rb"], nwp[:, 0:1])           # 1/(v.w)
            nc.vector.scalar_tensor_tensor(
                out=d["negstep"], in0=nwp[:, 1:2], scalar=-1.0, in1=d["rb"],
                op0=Alu.mult, op1=Alu.mult,
            )                                                    # -1/lmax
            nc.vector.reciprocal(d["bmax"], nwp[:, 1:2])         # 1/(v.v)
            nc.vector.scalar_tensor_tensor(
                out=d["invnegstep"], in0=nwp[:, 0:1], scalar=-1.0,
                in1=d["bmax"], op0=Alu.mult, op1=Alu.mult,
            )                                                    # -lmax

        # ---------- y0 = project(ones) ----------
        for h in (0, 1):
            d = H[h]
            nc.vector.memset(d["neglam"], -1e30)
            for st in newton_stages(h, d["imu"], d["mu"]):
                st()
            nc.vector.tensor_scalar(
                out=d["lam"], in0=d["neglam"], scalar1=-1.0, scalar2=None,
                op0=Alu.mult,
            )
            nc.vector.tensor_scalar(
                out=d["t"], in0=d["mu"], scalar1=d["lam"][:, 0:1], scalar2=1.0,
                op0=Alu.mult, op1=Alu.add,
            )
            nc.vector.tensor_scalar(
                out=d["ys"], in0=d["t"], scalar1=0.0,
                scalar2=d["negstep"][:, 0:1], op0=Alu.max, op1=Alu.mult,
            )
            transpose_to_xB(h, d["ys"])

        # ---------- PGD (software-pipelined halves, slotted schedule) ----------
        # Phase (k, h) streams matvec_h(k) in 8 fine stages; the OTHER half's
        # pending Newton (two independent pair-slice chains) is slotted
        # between stages so every PE instruction's inputs are ready well
        # before issue.  The pair1 transpose carries into the next phase.
        assert newton_k == 3, "PE slot plan below is laid out for newton_k=3"
        pending = {0: None, 1: None}   # half -> iteration awaiting update
        carry_tr = None                # (half, k) whose pair1 ys awaits tr
        for k in range(pgd_iters):
            for h in (0, 1):
                other = 1 - h
                ko = pending[other]
                c0 = c1 = None
                if ko is not None:
                    c0 = slice_update_stages(other, ko, 0)
                    c1 = slice_update_stages(other, ko, 1)
                mv = matvec_stages(h, H[h]["g"])
                if c0:
                    c0[0]()                     # pre+stt1 (p0) - DVE
                mv[0]()
                if carry_tr is not None:
                    tr_slice(carry_tr[0], 1, H[carry_tr[0]]["ys"])
                    carry_tr = None
                mv[1]()
                if c0:
                    c0[1]()                     # g1 p0
                mv[2]()
                if c1:
                    c1[0]()                     # pre+stt1 (p1) - DVE
                mv[3]()
                if c1:
                    c1[1]()                     # g1 p1
                mv[4]()
                if c0:
                    c0[2]()                     # g2 p0
                mv[5]()
                if c1:
                    c1[2]()                     # g2 p1
                if c0:
                    c0[3]()                     # g3 p0 (+post p0)
                mv[6]()
                if c0 and ko < pgd_iters - 1:
                    tr_slice(other, 0, H[other]["ys"])
                mv[7]()
                if c1:
                    c1[3]()                     # g3 p1 (+post p1)
                    if ko < pgd_iters - 1:
                        carry_tr = (other, ko)
                pending[other] = None
                pending[h] = k
        for h in (0, 1):
            if carry_tr is not None and carry_tr[0] == h:
                tr_slice(h, 1, H[h]["ys"])
                carry_tr = None
            if pending[h] is not None:
                for Pp in (0, 1):
                    for st in slice_update_stages(h, pending[h], Pp):
                        st()
                pending[h] = None

        # ---------- postprocess ----------
        for h in (0, 1):
            d = H[h]
            nc.vector.tensor_scalar(
                out=d["prod"][:, 0, :], in0=d["mu"], scalar1=1e-6, scalar2=None,
                op0=Alu.is_gt, op1=Alu.add, accum_out=d["cnt"],
            )
            cntp = nw_pool.tile([64, 2], F32, tag=f"nw{h}", name=f"nw{h}")
            gmm(h, d["cnt"], cntp, 1)
            nc.vector.tensor_scalar(
                out=d["mvd"], in0=cntp[:, 0:1], scalar1=0.5, scalar2=None,
                op0=Alu.is_gt,
            )
            nc.vector.tensor_scalar(
                out=d["omv"], in0=d["mvd"], scalar1=-1.0, scalar2=1.0,
                op0=Alu.mult, op1=Alu.add,
            )
            y2 = d["t"]   # reuse
            nc.vector.tensor_scalar(
                out=y2, in0=d["yfin"], scalar1=d["mvd"][:, 0:1],
                scalar2=d["omv"][:, 0:1], op0=Alu.mult, op1=Alu.add,
            )
            nc.vector.tensor_scalar(
                out=d["prod"][:, 0, :], in0=y2, scalar1=1.0, scalar2=None,
                op0=Alu.mult, op1=Alu.add, accum_out=d["sp"],
            )
            spp = nw_pool.tile([64, 2], F32, tag=f"nw{h}", name=f"nw{h}")
            gmm(h, d["sp"], spp, 1)
            nc.vector.tensor_scalar(
                out=d["ok"], in0=spp[:, 0:1], scalar1=1e-6, scalar2=None,
                op0=Alu.is_gt,
            )
            nc.vector.tensor_scalar(
                out=d["bmax"], in0=spp[:, 0:1], scalar1=1e-30, scalar2=None,
                op0=Alu.max,
            )
            nc.vector.reciprocal(d["rb"], d["bmax"])
            nc.vector.tensor_mul(d["sc"], d["rb"], d["ok"])
            nc.vector.tensor_scalar(
                out=d["off"], in0=d["ok"], scalar1=-1.0 / N, scalar2=1.0 / N,
                op0=Alu.mult, op1=Alu.add,
            )
            w1 = d["u"]   # reuse
            nc.vector.tensor_scalar(
                out=w1, in0=y2, scalar1=d["sc"][:, 0:1],
                scalar2=d["off"][:, 0:1], op0=Alu.mult, op1=Alu.add,
            )
            nc.vector.tensor_scalar(
                out=d["prod"][:, 0, :], in0=w1, scalar1=1.0, scalar2=None,
                op0=Alu.mult, op1=Alu.add, accum_out=d["s2"],
            )
            s2p = nw_pool.tile([64, 2], F32, tag=f"nw{h}", name=f"nw{h}")
            gmm(h, d["s2"], s2p, 1)
            nc.vector.reciprocal(d["rb"], s2p[:, 0:1])
            wf = d["r"]   # reuse
            nc.vector.tensor_scalar(
                out=wf, in0=w1, scalar1=d["rb"][:, 0:1], scalar2=None,
                op0=Alu.mult,
            )
            for Pp, s, dd, b in half_samples(h):
                p0, p1 = a4_sample_range(Pp, s, dd)
                nc.sync.dma_start(out=w_dram[b: b + 1, :],
                                  in_=wf[p0:p1, :])

    nc.compile()
    return nc


def _get_program():
    if "nc" not in _PROGRAM_CACHE:
        _PROGRAM_CACHE["nc"] = _build_program()
    return _PROGRAM_CACHE["nc"]


def _host_inputs(mu, sig):
    import ml_dtypes
    sig_bf = sig.astype(ml_dtypes.bfloat16)
    g64 = np.kron(np.eye(16, dtype=np.float32), np.ones((4, 4), np.float32))
    id64 = np.eye(64, dtype=np.float32)
    return sig_bf, g64, id64


def kernel(predicted_returns: np.ndarray, covariance_matrix: np.ndarray) -> np.ndarray:
    from concourse.bass_utils import run_bass_kernel_spmd

    mu = np.ascontiguousarray(predicted_returns, dtype=np.float32)
    sig = np.ascontiguousarray(covariance_matrix, dtype=np.float32)
    batch = mu.shape[0]
    assert batch == NCORES * SPC and mu.shape[1] == N

    sig_bf, g64, id64 = _host_inputs(mu, sig)

    nc = _get_program()
    in_maps = []
    for c in range(NCORES):
        sl = slice(c * SPC, (c + 1) * SPC)
        in_maps.append({
            "mu_in": mu[sl],
            "sigma_in": sig_bf[sl],
            "g64_in": g64,
            "id64_in": id64,
        })
    res = run_bass_kernel_spmd(nc, in_maps, core_ids=list(range(NCORES)))
    out = np.concatenate([r["w_out"] for r in res.results], axis=0)
    return out.astype(np.float32)


if __name__ == "__main__":
    rng = np.random.default_rng(0)
    mu = (0.05 + 0.1 * rng.random((NCORES * SPC, N))).astype(np.float32)
    A = rng.standard_normal((4, N, N)).astype(np.float32)
    sig = np.einsum("bik,bjk->bij", A, A) / N + 0.1 * np.eye(N, dtype=np.float32)
    sig = np.tile(sig, (64, 1, 1)).astype(np.float32)
    w = kernel(mu, sig)
    print(w.shape, w.sum(axis=1)[:4])
